# revision 17
# baseline (speedup 1.0000x reference)
"""DLPCNN loss (retrieval-kNN) on 8 Trainium2 NeuronCores via Bass/Tile.

Strategy (data-parallel over the batch, class-sorted, fp8 matmuls):
  - Host sorts rows by class; each of the 8 cores owns 256 contiguous sorted
    rows and a 128-aligned column window (wcol cols) covering the full class
    spans of its rows -- all valid same-class neighbors live in the window.
  - One augmented e4m3 matmul per core (DoubleRow perf mode: two 128-row
    k-tiles per instruction, 2x the bf16 rate) produces
      NM[i,j] = G - sq_j/2 - OFF*(1 - same_class)
    which ranks within a row exactly like -d2/2 (the sq_i/2 row constant
    drops out).  The lhsT for k-pairs 0..6 is a column slice of rt itself
    (x as both operands); only the last k-pair needs a separate tiny lta
    tensor carrying the asymmetric aug rows (sq splits with scale-32 lhsT,
    240*one-hot vs 8*one-hot giving the 1920 cross-class push-down).
    e4m3 max-normal is 240, so every constant is budgeted under it.
  - Per row: 21st-largest of NM (self included; self = sq_i/2 is the row
    max) via 3x (DVE max8 + match_replace); threshold -> 0/1 matrix A.
  - W' = A @ [x_w | split4(sq_w/32)] in fp8 DoubleRow gives neighbor-sum s'
    and neighbor-sq sum; ACT squares W' chunks for ||s'||^2.
  - SNM = sum of selected NM values (GpSimd scalar_tensor_tensor); host:
    sum(-d2) = 2*SNM - (K+1)*sq_i, then the same residual identities as
    the bf16 version reduce the loss to per-row scalars.
  - Device outputs per-row stats; host does the O(B) scalar reduction.

DMA: rt streams in k-pair groups on the SP queue (pacing mm1); the tiny
idt/pm/lta + the xa stream ride the ACT queue so descriptor-gen cost is
split across two sequencers.  A fence DMA on the ACT queue data-depends on
the last rt group so xa cannot steal HBM bandwidth from the mm1-pacing rt
stream.
"""

import sys

for _p in ("/opt/trn_rl_repo",):
    if _p not in sys.path:
        sys.path.insert(0, _p)

import numpy as np
import ml_dtypes

import concourse.bacc as bacc
import concourse.mybir as mybir
import concourse.tile as tile
from concourse.bass_utils import run_bass_kernel_spmd

B, D, C, K = 2048, 2000, 7, 20
LAMDA = 0.003
NCORES = 8
RPC = B // NCORES          # rows per core
MT = RPC // 128            # m-tiles per core
KR = 2048                  # augmented contraction rows (D data + 11 aug + pad)
KT = KR // 128
KT2 = KT // 2              # DoubleRow k-pairs
NSQ = 4                    # e4m3 split levels for sq rows/cols
NA = D + NSQ               # xa columns: [x | split4(sq/32)]
OFF = 1920.0               # cross-class push-down = 240 * 8 (e4m3 exact)
SQS = 32.0                 # scale for sq splits (exact power of 2)
NEG_FILL = -1.0e30

F32 = mybir.dt.float32
BF16 = mybir.dt.bfloat16
F8 = mybir.dt.float8e4
Alu = mybir.AluOpType
Act = mybir.ActivationFunctionType
Ax = mybir.AxisListType
DR = mybir.MatmulPerfMode.DoubleRow

NPF8 = ml_dtypes.float8_e4m3

_CACHE = {}

K2GROUPS = [(0, 1), (1, 3), (3, 6), (6, 8)]


def _chunks(total, step=512):
    return [(s, min(step, total - s)) for s in range(0, total, step)]


def _f8_split(v, levels, scale):
    """Split float64 vector v into `levels` e4m3 parts with scale*sum ~= v."""
    parts = []
    rem = v.astype(np.float64) / scale
    for _ in range(levels):
        p = rem.astype(NPF8)
        parts.append(p)
        rem = rem - p.astype(np.float64)
    return parts


def _build(wcol):
    """Each core's window is cyclically rolled on host so its own 256 rows
    sit at columns [0, 256) -- the NM lhsT is then the compile-time slice
    rt[:, k2, :, m*128:(m+1)*128] on every core."""
    wt = wcol // 128
    wt2 = wt // 2
    nc = bacc.Bacc("TRN2", target_bir_lowering=False, debug=False)
    rt_d = nc.dram_tensor("rt", [128, KT2, 2, wcol], F8, kind="ExternalInput").ap()
    lta_d = nc.dram_tensor("lta", [128, 2, RPC], F8, kind="ExternalInput").ap()
    xa_d = nc.dram_tensor("xa", [128, wt2, 2, NA], F8, kind="ExternalInput").ap()
    id_d = nc.dram_tensor("idt", [128, 128], BF16, kind="ExternalInput").ap()
    pm_d = nc.dram_tensor("pm", [128, MT, C], F32, kind="ExternalInput").ap()
    out_d = nc.dram_tensor("out", [128, 24], F32, kind="ExternalOutput").ap()

    with tile.TileContext(nc) as tc:
        with (
            tc.tile_pool(name="data", bufs=1) as data,
            tc.tile_pool(name="work", bufs=2) as work,
            tc.tile_pool(name="small", bufs=1) as small,
            tc.tile_pool(name="pnm", bufs=2, space="PSUM") as pnm,
            tc.tile_pool(name="pw", bufs=1, space="PSUM") as pw,
        ):
            # rt streams in k-pair groups on the SP queue (mm1 pacing).
            rt = data.tile([128, KT2, 2, wcol], F8)
            for (a, b) in K2GROUPS:
                nc.sync.dma_start(rt[:, a:b], rt_d[:, a:b])
            # tiny tensors + xa on the ACT queue (parallel descriptor gen)
            lta = small.tile([128, 2, RPC], F8)
            nc.scalar.dma_start(lta[:], lta_d[:])
            idt = small.tile([128, 128], BF16)
            nc.scalar.dma_start(idt[:], id_d[:])
            pmt = small.tile([128, MT, C], F32)
            nc.scalar.dma_start(pmt[:], pm_d[:])
            # ordering fence: this tiny SBUF->SBUF DMA data-depends on the
            # LAST rt k-group, so the xa trigger queued behind it on the ACT
            # sequencer cannot start streaming until the mm1-pacing rt
            # stream has fully landed (xa would otherwise steal ~half the
            # HBM bandwidth from the rt tail and push mm1 completion out)
            fence = small.tile([128, 1], F8)
            nc.scalar.dma_start(fence[:], rt[:, KT2 - 1, 1, 0:1])
            xa = data.tile([128, wt2, 2, NA], F8)
            nc.scalar.dma_start(xa[:], xa_d[:])

            outb = small.tile([128, 24], F32)
            atb = small.tile([128, wt2, 2, RPC], F8)   # A^T (fp8)

            # ---- CE pieces (independent; DVE is idle at kernel start) ----
            for m in range(MT):
                nc.vector.reduce_max(outb[:, 18 + m:19 + m], pmt[:, m, :], axis=Ax.X)
                negmx = work.tile([128, 1], F32)
                nc.gpsimd.tensor_scalar_mul(negmx[:], outb[:, 18 + m:19 + m], -1.0)
                e7 = work.tile([128, C], F32)
                nc.scalar.activation(
                    e7[:], pmt[:, m, :], Act.Exp, bias=negmx[:, 0:1], scale=1.0,
                    accum_out=outb[:, 20 + m:21 + m],
                )

            # ---- NM = G - sq_j/2 - OFF*(1-same)  (fp8 DoubleRow) ----
            # both m-tiles' matmuls are emitted BEFORE any top-k consumer:
            # engine streams are executed in program order, so this keeps PE
            # grinding mm1(m1) while DVE runs m0's top-k chain
            nms = []
            for m in range(MT):
                mo = m * 128
                nm = pnm.tile([128, wcol], F32, tag="nm", bufs=2, name=f"nm{m}")
                nms.append(nm)
                for (s, n) in _chunks(wcol):
                    for k2 in range(KT2):
                        lhsT = (
                            rt[:, k2, :, mo:mo + 128]
                            if k2 < KT2 - 1
                            else lta[:, :, m * 128:(m + 1) * 128]
                        )
                        nc.tensor.matmul(
                            nm[:, s:s + n],
                            lhsT=lhsT,
                            rhs=rt[:, k2, :, s:s + n],
                            start=(k2 == 0),
                            stop=(k2 == KT2 - 1),
                            perf_mode=DR,
                        )

            mns = []
            v3s = []
            for m in range(MT):
                ms = slice(m * 128, (m + 1) * 128)
                nm = nms[m]

                # single PSUM read; everything downstream reads the SBUF
                # copy. Copy on ACT: it is idle here and this keeps the
                # serial DVE top-k chain shorter
                mn = work.tile([128, wcol], F32)
                nc.scalar.copy(mn[:], nm[:])
                mns.append(mn)

                # ---- top-(K+1) threshold: 3 rounds of max8 ----
                v1 = work.tile([128, 8], F32)
                nc.vector.max(v1[:], mn[:])
                mn2 = work.tile([128, wcol], F32)
                nc.vector.match_replace(mn2[:], v1[:], mn[:], NEG_FILL)
                v2 = work.tile([128, 8], F32)
                nc.vector.max(v2[:], mn2[:])
                mn3 = work.tile([128, wcol], F32)
                nc.vector.match_replace(mn3[:], v2[:], mn2[:], NEG_FILL)
                v3 = work.tile([128, 8], F32)
                nc.vector.max(v3[:], mn3[:])
                v3s.append(v3)

                # A = (NM >= t) as bf16 (ptr-scalar ops are DVE-only);
                # fp8 PE transpose needs stride-2 PSUM writes, so transpose
                # in bf16 and cast to fp8 in the PSUM->SBUF copy instead
                abh = work.tile([128, wcol], BF16)
                nc.vector.tensor_scalar(abh[:], mn[:], v3[:, 4:5], None, op0=Alu.is_ge)
                for t in range(wt):
                    tr = pnm.tile([128, 128], BF16, tag="nm", bufs=2, name=f"tr{m}_{t}")
                    nc.tensor.transpose(tr[:], abh[:, t * 128:(t + 1) * 128], idt[:])
                    if t % 2 == 0:
                        nc.vector.tensor_copy(atb[:, t // 2, t % 2, ms], tr[:])
                    else:
                        nc.scalar.copy(atb[:, t // 2, t % 2, ms], tr[:])

                # ---- W' = A @ [x_w | split4(sq_w/32)]  (fp8 DoubleRow) ----
                # one single-bank PSUM tile per 512-chunk so each chunk's
                # matmul group is independent of the others' square-reduces
                for ci, (s, n) in enumerate(_chunks(NA)):
                    w = pw.tile([128, n], F32, tag=f"w{ci}", name=f"w{m}_{ci}")
                    for t2 in range(wt2):
                        nc.tensor.matmul(
                            w[:],
                            lhsT=atb[:, t2, :, ms],
                            rhs=xa[:, t2, :, s:s + n],
                            start=(t2 == 0),
                            stop=(t2 == wt2 - 1),
                            perf_mode=DR,
                        )
                    # pipelined ||s'||^2: square-reduce each chunk as soon as
                    # its accumulation group completes (exclude the sq cols)
                    ne = min(s + n, D) - s
                    sq2 = work.tile([128, 512], BF16, tag="sq2")
                    nc.scalar.activation(
                        sq2[:, :ne], w[:, :ne], Act.Square,
                        accum_out=outb[:, 10 + 4 * m + ci:11 + 4 * m + ci],
                    )
                    if s + n > D:
                        lo = D - s
                        nc.vector.tensor_copy(
                            outb[:, 2 + 4 * m:6 + 4 * m], w[:, lo:lo + NSQ]
                        )

            # deferred SNM reduces (off the critical top-k chain)
            for m in range(MT):
                scr = work.tile([128, wcol], F32)
                nc.vector.scalar_tensor_tensor(
                    out=scr[:], in0=mns[m][:], scalar=v3s[m][:, 4:5],
                    in1=mns[m][:],
                    op0=Alu.is_ge, op1=Alu.mult,
                    accum_out=outb[:, m:m + 1],
                )

            nc.sync.dma_start(out_d[:], outb[:])

    nc.compile()
    return nc


def _plan_windows(ys):
    """Per-core window [ws, ws+wcol) covering the full class spans of the
    core's rows.  The window is later rolled so the core's own rows sit at
    columns [0, 256); only hi-lo <= wcol matters here, not alignment."""
    starts_c = np.searchsorted(ys, np.arange(C))
    ends_c = np.searchsorted(ys, np.arange(C), side="right")
    need = []
    for c in range(NCORES):
        blo, bhi = c * RPC, (c + 1) * RPC
        cls = np.unique(ys[blo:bhi])
        lo = int(min(starts_c[k] for k in cls))
        hi = int(max(ends_c[k] for k in cls))
        need.append((lo, hi))
    wneed = max(hi - lo for lo, hi in need)
    wcol = 256 * ((max(wneed, 512) + 255) // 256)  # even # of 128-tiles
    starts = []
    for c, (lo, hi) in enumerate(need):
        ws = min(lo, B - wcol)
        assert ws + wcol >= hi and 0 <= ws <= c * RPC, (c, ws, wcol, lo, hi)
        starts.append(ws)
    return wcol, starts


def kernel(preds, x, y):
    y = np.asarray(y).astype(np.int64)
    preds = np.ascontiguousarray(np.asarray(preds, dtype=np.float32))
    x = np.ascontiguousarray(np.asarray(x, dtype=np.float32))
    assert x.shape == (B, D) and preds.shape == (B, C) and y.shape == (B,)

    order = np.argsort(y, kind="stable")
    xs = x[order]
    ys = y[order]
    ps = preds[order]
    sq64 = np.einsum("ij,ij->i", xs.astype(np.float64), xs.astype(np.float64))
    sq = sq64.astype(np.float32)

    wcol, starts = _plan_windows(ys)
    cls_count = np.bincount(ys, minlength=C)
    assert (cls_count >= K + 1).all(), cls_count

    oh = np.zeros((C, B), np.float32)
    oh[ys, np.arange(B)] = 1.0

    # global augmented rhs for the NM matmul [KR, B] in e4m3:
    #   rows 0..D-1: x^T ; D..D+3: split4((-sq/2 - OFF)/32) with lhsT 32
    #   D+4..D+10: 8*one-hot(class) with lhsT 240*one-hot ; rest zero
    rhs_g = np.zeros((KR, B), NPF8)
    rhs_g[:D] = xs.T.astype(NPF8)
    qparts = _f8_split(-(sq64 / 2.0 + OFF), NSQ, SQS)
    for t in range(NSQ):
        rhs_g[D + t] = qparts[t]
    rhs_g[D + NSQ:D + NSQ + C] = (8.0 * oh).astype(NPF8)
    # partition-major swizzle [KR, B] -> [128, KT, B]
    rhs_gp = np.ascontiguousarray(rhs_g.reshape(KT, 128, B).transpose(1, 0, 2))

    xa_g = np.zeros((B, NA), NPF8)
    xa_g[:, :D] = xs.astype(NPF8)
    sparts = _f8_split(sq64, NSQ, SQS)
    for t in range(NSQ):
        xa_g[:, D + t] = sparts[t]

    if wcol not in _CACHE:
        _CACHE[wcol] = _build(wcol)
    nc = _CACHE[wcol]
    wt = wcol // 128

    # lta: lhsT for the LAST DoubleRow k-pair (k-tiles 14,15) of each core.
    # [p, 0, i] = feature row 1792+p of x_i ; [p, 1, i]: features 1920..1999
    # for p<80, then 32 (sq-split scale) for p in 80..83, then 240*one-hot.
    in_maps = []
    for cidx in range(NCORES):
        my = slice(cidx * RPC, (cidx + 1) * RPC)
        ws = starts[cidx]
        roll = cidx * RPC - ws   # my rows sit at window cols [roll, roll+256)
        lta = np.zeros((128, 2, RPC), NPF8)
        lta[:, 0, :] = xs[my, 1792:1920].T.astype(NPF8)
        lta[:80, 1, :] = xs[my, 1920:2000].T.astype(NPF8)
        lta[80:80 + NSQ, 1, :] = np.float32(SQS)
        lta[80 + NSQ:80 + NSQ + C, 1, :] = (240.0 * oh[:, my]).astype(NPF8)
        rtw = np.roll(rhs_gp[:, :, ws:ws + wcol], -roll, axis=2)
        xaw = np.roll(xa_g[ws:ws + wcol], -roll, axis=0)
        in_maps.append({
            "rt": np.ascontiguousarray(rtw.reshape(128, KT2, 2, wcol)),
            "lta": lta,
            "xa": np.ascontiguousarray(
                xaw.reshape(wt // 2, 2, 128, NA).transpose(2, 0, 1, 3)),
            "idt": np.eye(128, dtype=ml_dtypes.bfloat16),
            "pm": np.ascontiguousarray(
                ps[my].reshape(MT, 128, C).transpose(1, 0, 2)),
        })

    res = run_bass_kernel_spmd(nc, in_maps, core_ids=list(range(NCORES)))

    # host-side unshard: per-row stats -> two scalar loss terms
    lp_sum = 0.0
    ce_sum = 0.0
    for cidx in range(NCORES):
        my = slice(cidx * RPC, (cidx + 1) * RPC)
        o = res.results[cidx]["out"].astype(np.float64)
        sq_my = sq64[my]
        # snm_meas = sum_sel (G - sq_j/2);  sum_sel(-d2) = 2*snm - (K+1)*sq_i
        snm = 2.0 * np.stack([o[:, 0], o[:, 1]]).reshape(RPC) - (K + 1) * sq_my
        ssq = SQS * np.stack(
            [o[:, 2:6].sum(1), o[:, 6:10].sum(1)]).reshape(RPC)
        ssn = np.stack([o[:, 10:14].sum(1), o[:, 14:18].sum(1)]).reshape(RPC)
        mx = o[:, 18:20].T.reshape(RPC)
        se = o[:, 20:22].T.reshape(RPC)
        gp = 0.5 * (snm + (K + 1) * sq_my + ssq)
        lp = sq_my - (2.0 / K) * (gp - sq_my) + (ssn - 2.0 * gp + sq_my) / K**2
        lp_sum += lp.sum()
        lse = np.log(se) + mx
        pick = ps[my][np.arange(RPC), ys[my]].astype(np.float64)
        ce_sum += (lse - pick).sum()

    loss = LAMDA * (lp_sum / B) / 2.0 + ce_sum / B
    return np.float32(loss)


# revision 24
# speedup vs baseline: 1.1616x; 1.1616x over previous
"""DLPCNN loss (retrieval-kNN) on 8 Trainium2 NeuronCores via Bass/Tile.

Strategy (data-parallel over the batch, class-sorted, fp8 matmuls):
  - Host sorts rows by class; each of the 8 cores owns 256 contiguous sorted
    rows and a 128-aligned column window (wcol cols) covering the full class
    spans of its rows -- all valid same-class neighbors live in the window.
  - One augmented e4m3 matmul per core (DoubleRow perf mode: two 128-row
    k-tiles per instruction, 2x the bf16 rate) produces
      NM[i,j] = G - sq_j/2 - OFF*(1 - same_class)
    which ranks within a row exactly like -d2/2 (the sq_i/2 row constant
    drops out).  The lhsT for k-pairs 0..6 is a column slice of rt itself
    (x as both operands); only the last k-pair needs a separate tiny lta
    tensor carrying the asymmetric aug rows (sq splits with scale-32 lhsT,
    240*one-hot vs 8*one-hot giving the 1920 cross-class push-down).
    e4m3 max-normal is 240, so every constant is budgeted under it.
  - Per row: 21st-largest of NM (self included; self = sq_i/2 is the row
    max) via 3x (DVE max8 + match_replace); threshold -> 0/1 matrix A.
  - W' = A @ [x_w | split4(sq_w/32)] in fp8 DoubleRow gives neighbor-sum s'
    and neighbor-sq sum; ACT squares W' chunks for ||s'||^2.
  - SNM = sum of selected NM values (GpSimd scalar_tensor_tensor); host:
    sum(-d2) = 2*SNM - (K+1)*sq_i, then the same residual identities as
    the bf16 version reduce the loss to per-row scalars.
  - Device outputs per-row stats; host does the O(B) scalar reduction.

DMA: rt streams in k-pair groups on the SP queue (pacing mm1); the tiny
idt/pm/lta + the xa stream ride the ACT queue so descriptor-gen cost is
split across two sequencers.  A fence DMA on the ACT queue data-depends on
the last rt group so xa cannot steal HBM bandwidth from the mm1-pacing rt
stream.
"""

import sys

for _p in ("/opt/trn_rl_repo",):
    if _p not in sys.path:
        sys.path.insert(0, _p)

import numpy as np
import ml_dtypes

import concourse.bacc as bacc
import concourse.mybir as mybir
import concourse.tile as tile
from concourse.tile import add_dep_helper
from concourse.bass_utils import run_bass_kernel_spmd

B, D, C, K = 2048, 2000, 7, 20
LAMDA = 0.003
NCORES = 8
RPC = B // NCORES          # rows per core
MT = RPC // 128            # m-tiles per core
KR = 2048                  # augmented contraction rows (D data + 11 aug + pad)
KT = KR // 128
KT2 = KT // 2              # DoubleRow k-pairs
NSQ = 4                    # e4m3 split levels for sq rows/cols
NA = D + NSQ               # xa columns: [x | split4(sq/32)]
OFF = 1920.0               # cross-class push-down = 240 * 8 (e4m3 exact)
SQS = 32.0                 # scale for sq splits (exact power of 2)
NEG_FILL = -1.0e30

F32 = mybir.dt.float32
BF16 = mybir.dt.bfloat16
F8 = mybir.dt.float8e4
Alu = mybir.AluOpType
Act = mybir.ActivationFunctionType
Ax = mybir.AxisListType
DR = mybir.MatmulPerfMode.DoubleRow

NPF8 = ml_dtypes.float8_e4m3

_CACHE = {}

K2GROUPS = [(0, 1), (1, 3), (3, 6), (6, 8)]


def _chunks(total, step=512):
    return [(s, min(step, total - s)) for s in range(0, total, step)]


def _f8_split(v, levels, scale):
    """Split float64 vector v into `levels` e4m3 parts with scale*sum ~= v."""
    parts = []
    rem = v.astype(np.float64) / scale
    for _ in range(levels):
        p = rem.astype(NPF8)
        parts.append(p)
        rem = rem - p.astype(np.float64)
    return parts


def _build(wcol):
    """Each core's window is cyclically rolled on host so its own 256 rows
    sit at columns [0, 256) -- the NM lhsT is then the compile-time slice
    rt[:, k2, :, m*128:(m+1)*128] on every core."""
    wt = wcol // 128
    wt2 = wt // 2
    nc = bacc.Bacc("TRN2", target_bir_lowering=False, debug=False)
    rt_d = nc.dram_tensor("rt", [128, KT2, 2, wcol], F8, kind="ExternalInput").ap()
    lta_d = nc.dram_tensor("lta", [128, 2, RPC], F8, kind="ExternalInput").ap()
    xa_d = nc.dram_tensor("xa", [128, wt2, 2, NA], F8, kind="ExternalInput").ap()
    id_d = nc.dram_tensor("idt", [128, 128], BF16, kind="ExternalInput").ap()
    pm_d = nc.dram_tensor("pm", [128, MT, C], F32, kind="ExternalInput").ap()
    out_d = nc.dram_tensor("out", [128, 24], F32, kind="ExternalOutput").ap()

    with tile.TileContext(nc) as tc:
        with (
            tc.tile_pool(name="data", bufs=1) as data,
            tc.tile_pool(name="work", bufs=2) as work,
            tc.tile_pool(name="small", bufs=1) as small,
            tc.tile_pool(name="pnm", bufs=2, space="PSUM") as pnm,
            tc.tile_pool(name="pw", bufs=1, space="PSUM") as pw,
        ):
            # rt streams in k-pair groups on the SP queue (mm1 pacing).
            rt = data.tile([128, KT2, 2, wcol], F8)
            for (a, b) in K2GROUPS:
                nc.sync.dma_start(rt[:, a:b], rt_d[:, a:b])
            # tiny tensors on the ACT queue (parallel descriptor gen); the
            # fence/xa MUST NOT ride the ACT queue -- the fence stalls its
            # sequencer until rt lands, and ACT has PSUM->SBUF copies to do
            lta = small.tile([128, 2, RPC], F8)
            nc.scalar.dma_start(lta[:], lta_d[:])
            idt = small.tile([128, 128], BF16)
            nc.scalar.dma_start(idt[:], id_d[:])
            pmt = small.tile([128, MT, C], F32)
            nc.scalar.dma_start(pmt[:], pm_d[:])
            # ordering fence: this tiny SBUF->SBUF DMA data-depends on the
            # LAST rt k-group, so the xa trigger queued behind it on the SP
            # sequencer cannot start streaming until the mm1-pacing rt
            # stream has fully landed (xa would otherwise steal ~half the
            # HBM bandwidth from the rt tail and push mm1 completion out)
            fence = small.tile([128, 1], F8)
            nc.sync.dma_start(fence[:], rt[:, KT2 - 1, 1, 0:1])
            xa = data.tile([128, wt2, 2, NA], F8)
            nc.sync.dma_start(xa[:], xa_d[:])

            outb = small.tile([128, 24], F32)
            atb = small.tile([128, wt2, 2, RPC], F8)   # A^T (fp8)

            # ---- CE pieces (independent; DVE is idle at kernel start) ----
            for m in range(MT):
                nc.vector.reduce_max(outb[:, 18 + m:19 + m], pmt[:, m, :], axis=Ax.X)
                negmx = work.tile([128, 1], F32)
                nc.gpsimd.tensor_scalar_mul(negmx[:], outb[:, 18 + m:19 + m], -1.0)
                e7 = work.tile([128, C], F32)
                nc.scalar.activation(
                    e7[:], pmt[:, m, :], Act.Exp, bias=negmx[:, 0:1], scale=1.0,
                    accum_out=outb[:, 20 + m:21 + m],
                )

            # ---- NM = G - sq_j/2 - OFF*(1-same)  (fp8 DoubleRow) ----
            # both m-tiles' matmuls are emitted BEFORE any top-k consumer:
            # engine streams are executed in program order, so this keeps PE
            # grinding mm1(m1) while DVE runs m0's top-k chain
            nms = []
            for m in range(MT):
                mo = m * 128
                nm = pnm.tile([128, wcol], F32, tag="nm", bufs=2, name=f"nm{m}")
                nms.append(nm)
                for (s, n) in _chunks(wcol):
                    for k2 in range(KT2):
                        lhsT = (
                            rt[:, k2, :, mo:mo + 128]
                            if k2 < KT2 - 1
                            else lta[:, :, m * 128:(m + 1) * 128]
                        )
                        nc.tensor.matmul(
                            nm[:, s:s + n],
                            lhsT=lhsT,
                            rhs=rt[:, k2, :, s:s + n],
                            start=(k2 == 0),
                            stop=(k2 == KT2 - 1),
                            perf_mode=DR,
                        )

            mns = []
            v3s = []
            ige_insts = []
            for m in range(MT):
                ms = slice(m * 128, (m + 1) * 128)
                nm = nms[m]

                # single PSUM read; everything downstream reads the SBUF
                # copy. Copy on ACT: it is idle here and this keeps the
                # serial DVE top-k chain shorter
                mn = work.tile([128, wcol], F32)
                nc.scalar.copy(mn[:], nm[:])
                mns.append(mn)

                # ---- top-(K+1) threshold: 3 rounds of max8 ----
                v1 = work.tile([128, 8], F32)
                mx = nc.vector.max(v1[:], mn[:])
                if m > 0:
                    # m1's chain strictly AFTER m0's threshold: the list
                    # scheduler otherwise interleaves the two serial DVE
                    # chains and m0's A-matrix lands ~5us late, stalling PE
                    add_dep_helper(mx.ins, ige_insts[m - 1].ins,
                                   reason="serialize m-tile top-k chains")
                mn2 = work.tile([128, wcol], F32)
                nc.vector.match_replace(mn2[:], v1[:], mn[:], NEG_FILL)
                v2 = work.tile([128, 8], F32)
                nc.vector.max(v2[:], mn2[:])
                mn3 = work.tile([128, wcol], F32)
                nc.vector.match_replace(mn3[:], v2[:], mn2[:], NEG_FILL)
                v3 = work.tile([128, 8], F32)
                nc.vector.max(v3[:], mn3[:])
                v3s.append(v3)

                # A = (NM >= t) as bf16 (ptr-scalar ops are DVE-only);
                # fp8 PE transpose needs stride-2 PSUM writes, so transpose
                # in bf16 and cast to fp8 in the PSUM->SBUF copy instead
                abh = work.tile([128, wcol], BF16)
                ige = nc.vector.tensor_scalar(
                    abh[:], mn[:], v3[:, 4:5], None, op0=Alu.is_ge)
                ige_insts.append(ige)
                for t in range(wt):
                    tr = pnm.tile([128, 128], BF16, tag="nm", bufs=2, name=f"tr{m}_{t}")
                    nc.tensor.transpose(tr[:], abh[:, t * 128:(t + 1) * 128], idt[:])
                    nc.scalar.copy(atb[:, t // 2, t % 2, ms], tr[:])

                # ---- W' = A @ [x_w | split4(sq_w/32)]  (fp8 DoubleRow) ----
                # one single-bank PSUM tile per 512-chunk so each chunk's
                # matmul group is independent of the others' square-reduces
                for ci, (s, n) in enumerate(_chunks(NA)):
                    w = pw.tile([128, n], F32, tag=f"w{ci}", name=f"w{m}_{ci}")
                    for t2 in range(wt2):
                        nc.tensor.matmul(
                            w[:],
                            lhsT=atb[:, t2, :, ms],
                            rhs=xa[:, t2, :, s:s + n],
                            start=(t2 == 0),
                            stop=(t2 == wt2 - 1),
                            perf_mode=DR,
                        )
                    # pipelined ||s'||^2: square-reduce each chunk as soon
                    # as its group completes (exclude the sq cols)
                    ne = min(s + n, D) - s
                    acc = outb[:, 10 + 4 * m + ci:11 + 4 * m + ci]
                    sq2 = work.tile([128, 512], BF16, tag="sq2")
                    nc.scalar.activation(
                        sq2[:, :ne], w[:, :ne], Act.Square, accum_out=acc)
                    if s + n > D:
                        lo = D - s
                        nc.scalar.copy(
                            outb[:, 2 + 4 * m:6 + 4 * m], w[:, lo:lo + NSQ]
                        )

            # deferred SNM reduces (off the critical top-k chain)
            for m in range(MT):
                scr = work.tile([128, wcol], F32)
                stt = nc.vector.scalar_tensor_tensor(
                    out=scr[:], in0=mns[m][:], scalar=v3s[m][:, 4:5],
                    in1=mns[m][:],
                    op0=Alu.is_ge, op1=Alu.mult,
                    accum_out=outb[:, m:m + 1],
                )
                # keep DVE on the m1 top-k chain until both A-matrices exist
                add_dep_helper(stt.ins, ige_insts[-1].ins,
                               reason="SNM reduces after last is_ge")

            nc.sync.dma_start(out_d[:], outb[:])

    nc.compile()
    return nc


def _plan_windows(ys):
    """Per-core window [ws, ws+wcol) covering the full class spans of the
    core's rows.  The window is later rolled so the core's own rows sit at
    columns [0, 256); only hi-lo <= wcol matters here, not alignment."""
    starts_c = np.searchsorted(ys, np.arange(C))
    ends_c = np.searchsorted(ys, np.arange(C), side="right")
    need = []
    for c in range(NCORES):
        blo, bhi = c * RPC, (c + 1) * RPC
        cls = np.unique(ys[blo:bhi])
        lo = int(min(starts_c[k] for k in cls))
        hi = int(max(ends_c[k] for k in cls))
        need.append((lo, hi))
    wneed = max(hi - lo for lo, hi in need)
    wcol = 256 * ((max(wneed, 512) + 255) // 256)  # even # of 128-tiles
    starts = []
    for c, (lo, hi) in enumerate(need):
        ws = min(lo, B - wcol)
        assert ws + wcol >= hi and 0 <= ws <= c * RPC, (c, ws, wcol, lo, hi)
        starts.append(ws)
    return wcol, starts


def kernel(preds, x, y):
    y = np.asarray(y).astype(np.int64)
    preds = np.ascontiguousarray(np.asarray(preds, dtype=np.float32))
    x = np.ascontiguousarray(np.asarray(x, dtype=np.float32))
    assert x.shape == (B, D) and preds.shape == (B, C) and y.shape == (B,)

    order = np.argsort(y, kind="stable")
    xs = x[order]
    ys = y[order]
    ps = preds[order]
    sq64 = np.einsum("ij,ij->i", xs.astype(np.float64), xs.astype(np.float64))
    sq = sq64.astype(np.float32)

    wcol, starts = _plan_windows(ys)
    cls_count = np.bincount(ys, minlength=C)
    assert (cls_count >= K + 1).all(), cls_count

    oh = np.zeros((C, B), np.float32)
    oh[ys, np.arange(B)] = 1.0

    # global augmented rhs for the NM matmul [KR, B] in e4m3:
    #   rows 0..D-1: x^T ; D..D+3: split4((-sq/2 - OFF)/32) with lhsT 32
    #   D+4..D+10: 8*one-hot(class) with lhsT 240*one-hot ; rest zero
    rhs_g = np.zeros((KR, B), NPF8)
    rhs_g[:D] = xs.T.astype(NPF8)
    qparts = _f8_split(-(sq64 / 2.0 + OFF), NSQ, SQS)
    for t in range(NSQ):
        rhs_g[D + t] = qparts[t]
    rhs_g[D + NSQ:D + NSQ + C] = (8.0 * oh).astype(NPF8)
    # partition-major swizzle [KR, B] -> [128, KT, B]
    rhs_gp = np.ascontiguousarray(rhs_g.reshape(KT, 128, B).transpose(1, 0, 2))

    xa_g = np.zeros((B, NA), NPF8)
    xa_g[:, :D] = xs.astype(NPF8)
    sparts = _f8_split(sq64, NSQ, SQS)
    for t in range(NSQ):
        xa_g[:, D + t] = sparts[t]

    if wcol not in _CACHE:
        _CACHE[wcol] = _build(wcol)
    nc = _CACHE[wcol]
    wt = wcol // 128

    # lta: lhsT for the LAST DoubleRow k-pair (k-tiles 14,15) of each core.
    # [p, 0, i] = feature row 1792+p of x_i ; [p, 1, i]: features 1920..1999
    # for p<80, then 32 (sq-split scale) for p in 80..83, then 240*one-hot.
    in_maps = []
    for cidx in range(NCORES):
        my = slice(cidx * RPC, (cidx + 1) * RPC)
        ws = starts[cidx]
        roll = cidx * RPC - ws   # my rows sit at window cols [roll, roll+256)
        lta = np.zeros((128, 2, RPC), NPF8)
        lta[:, 0, :] = xs[my, 1792:1920].T.astype(NPF8)
        lta[:80, 1, :] = xs[my, 1920:2000].T.astype(NPF8)
        lta[80:80 + NSQ, 1, :] = np.float32(SQS)
        lta[80 + NSQ:80 + NSQ + C, 1, :] = (240.0 * oh[:, my]).astype(NPF8)
        rtw = np.roll(rhs_gp[:, :, ws:ws + wcol], -roll, axis=2)
        xaw = np.roll(xa_g[ws:ws + wcol], -roll, axis=0)
        in_maps.append({
            "rt": np.ascontiguousarray(rtw.reshape(128, KT2, 2, wcol)),
            "lta": lta,
            "xa": np.ascontiguousarray(
                xaw.reshape(wt // 2, 2, 128, NA).transpose(2, 0, 1, 3)),
            "idt": np.eye(128, dtype=ml_dtypes.bfloat16),
            "pm": np.ascontiguousarray(
                ps[my].reshape(MT, 128, C).transpose(1, 0, 2)),
        })

    res = run_bass_kernel_spmd(nc, in_maps, core_ids=list(range(NCORES)))

    # host-side unshard: per-row stats -> two scalar loss terms
    lp_sum = 0.0
    ce_sum = 0.0
    for cidx in range(NCORES):
        my = slice(cidx * RPC, (cidx + 1) * RPC)
        o = res.results[cidx]["out"].astype(np.float64)
        sq_my = sq64[my]
        # snm_meas = sum_sel (G - sq_j/2);  sum_sel(-d2) = 2*snm - (K+1)*sq_i
        snm = 2.0 * np.stack([o[:, 0], o[:, 1]]).reshape(RPC) - (K + 1) * sq_my
        ssq = SQS * np.stack(
            [o[:, 2:6].sum(1), o[:, 6:10].sum(1)]).reshape(RPC)
        ssn = np.stack([o[:, 10:14].sum(1), o[:, 14:18].sum(1)]).reshape(RPC)
        mx = o[:, 18:20].T.reshape(RPC)
        se = o[:, 20:22].T.reshape(RPC)
        gp = 0.5 * (snm + (K + 1) * sq_my + ssq)
        lp = sq_my - (2.0 / K) * (gp - sq_my) + (ssn - 2.0 * gp + sq_my) / K**2
        lp_sum += lp.sum()
        lse = np.log(se) + mx
        pick = ps[my][np.arange(RPC), ys[my]].astype(np.float64)
        ce_sum += (lse - pick).sum()

    loss = LAMDA * (lp_sum / B) / 2.0 + ce_sum / B
    return np.float32(loss)


# revision 28
# speedup vs baseline: 1.1671x; 1.0048x over previous
"""DLPCNN loss (retrieval-kNN) on 8 Trainium2 NeuronCores via Bass/Tile.

Strategy (data-parallel over the batch, class-sorted, fp8 matmuls):
  - Host sorts rows by class; each of the 8 cores owns 256 contiguous sorted
    rows and a 128-aligned column window (wcol cols) covering the full class
    spans of its rows -- all valid same-class neighbors live in the window.
  - One augmented e4m3 matmul per core (DoubleRow perf mode: two 128-row
    k-tiles per instruction, 2x the bf16 rate) produces
      NM[i,j] = G - sq_j/2 - OFF*(1 - same_class)
    which ranks within a row exactly like -d2/2 (the sq_i/2 row constant
    drops out).  The lhsT for k-pairs 0..6 is a column slice of rt itself
    (x as both operands); only the last k-pair needs a separate tiny lta
    tensor carrying the asymmetric aug rows (sq splits with scale-32 lhsT,
    240*one-hot vs 8*one-hot giving the 1920 cross-class push-down).
    e4m3 max-normal is 240, so every constant is budgeted under it.
  - Per row: 21st-largest of NM (self included; self = sq_i/2 is the row
    max) via 3x (DVE max8 + match_replace); threshold -> 0/1 matrix A.
  - W' = A @ [x_w | split4(sq_w/32)] in fp8 DoubleRow gives neighbor-sum s'
    and neighbor-sq sum; ACT squares W' chunks for ||s'||^2.
  - SNM = sum of selected NM values (GpSimd scalar_tensor_tensor); host:
    sum(-d2) = 2*SNM - (K+1)*sq_i, then the same residual identities as
    the bf16 version reduce the loss to per-row scalars.
  - Device outputs per-row stats; host does the O(B) scalar reduction.

DMA: rt streams in k-pair groups on the SP queue (pacing mm1); the tiny
idt/pm/lta + the xa stream ride the ACT queue so descriptor-gen cost is
split across two sequencers.  A fence DMA on the ACT queue data-depends on
the last rt group so xa cannot steal HBM bandwidth from the mm1-pacing rt
stream.
"""

import sys

for _p in ("/opt/trn_rl_repo",):
    if _p not in sys.path:
        sys.path.insert(0, _p)

import numpy as np
import ml_dtypes

import concourse.bacc as bacc
import concourse.mybir as mybir
import concourse.tile as tile
from concourse.tile import add_dep_helper
from concourse.bass_utils import run_bass_kernel_spmd

B, D, C, K = 2048, 2000, 7, 20
LAMDA = 0.003
NCORES = 8
RPC = B // NCORES          # rows per core
MT = RPC // 128            # m-tiles per core
KR = 2048                  # augmented contraction rows (D data + 11 aug + pad)
KT = KR // 128
KT2 = KT // 2              # DoubleRow k-pairs
NSQ = 4                    # e4m3 split levels for sq rows/cols
NA = D + NSQ               # xa columns: [x | split4(sq/32)]
OFF = 1920.0               # cross-class push-down = 240 * 8 (e4m3 exact)
SQS = 32.0                 # scale for sq splits (exact power of 2)
NEG_FILL = -1.0e30

F32 = mybir.dt.float32
BF16 = mybir.dt.bfloat16
F8 = mybir.dt.float8e4
Alu = mybir.AluOpType
Act = mybir.ActivationFunctionType
Ax = mybir.AxisListType
DR = mybir.MatmulPerfMode.DoubleRow

NPF8 = ml_dtypes.float8_e4m3

_CACHE = {}

K2GROUPS = [(0, 1), (1, 2), (2, 4), (4, 8)]


def _chunks(total, step=512):
    return [(s, min(step, total - s)) for s in range(0, total, step)]


def _f8_split(v, levels, scale):
    """Split float64 vector v into `levels` e4m3 parts with scale*sum ~= v."""
    parts = []
    rem = v.astype(np.float64) / scale
    for _ in range(levels):
        p = rem.astype(NPF8)
        parts.append(p)
        rem = rem - p.astype(np.float64)
    return parts


def _build(wcol):
    """Each core's window is cyclically rolled on host so its own 256 rows
    sit at columns [0, 256) -- the NM lhsT is then the compile-time slice
    rt[:, k2, :, m*128:(m+1)*128] on every core."""
    wt = wcol // 128
    wt2 = wt // 2
    nc = bacc.Bacc("TRN2", target_bir_lowering=False, debug=False)
    rt_d = nc.dram_tensor("rt", [128, KT2, 2, wcol], F8, kind="ExternalInput").ap()
    lta_d = nc.dram_tensor("lta", [128, 2, RPC], F8, kind="ExternalInput").ap()
    xa_d = nc.dram_tensor("xa", [128, wt2, 2, NA], F8, kind="ExternalInput").ap()
    id_d = nc.dram_tensor("idt", [128, 128], BF16, kind="ExternalInput").ap()
    pm_d = nc.dram_tensor("pm", [128, MT, C], F32, kind="ExternalInput").ap()
    out_d = nc.dram_tensor("out", [128, 24], F32, kind="ExternalOutput").ap()

    with tile.TileContext(nc) as tc:
        with (
            tc.tile_pool(name="data", bufs=1) as data,
            tc.tile_pool(name="work", bufs=2) as work,
            tc.tile_pool(name="small", bufs=1) as small,
            tc.tile_pool(name="pnm", bufs=2, space="PSUM") as pnm,
            tc.tile_pool(name="pw", bufs=1, space="PSUM") as pw,
        ):
            # rt streams in k-pair groups on the SP queue (mm1 pacing).
            rt = data.tile([128, KT2, 2, wcol], F8)
            for (a, b) in K2GROUPS:
                nc.sync.dma_start(rt[:, a:b], rt_d[:, a:b])
            # tiny tensors on the ACT queue (parallel descriptor gen); the
            # fence/xa MUST NOT ride the ACT queue -- the fence stalls its
            # sequencer until rt lands, and ACT has PSUM->SBUF copies to do
            lta = small.tile([128, 2, RPC], F8)
            nc.scalar.dma_start(lta[:], lta_d[:])
            idt = small.tile([128, 128], BF16)
            nc.scalar.dma_start(idt[:], id_d[:])
            pmt = small.tile([128, MT, C], F32)
            nc.scalar.dma_start(pmt[:], pm_d[:])
            # ordering fence: this tiny SBUF->SBUF DMA data-depends on the
            # LAST rt k-group, so the xa trigger queued behind it on the SP
            # sequencer cannot start streaming until the mm1-pacing rt
            # stream has fully landed (xa would otherwise steal ~half the
            # HBM bandwidth from the rt tail and push mm1 completion out)
            fence = small.tile([128, 1], F8)
            nc.sync.dma_start(fence[:], rt[:, KT2 - 1, 1, 0:1])
            xa = data.tile([128, wt2, 2, NA], F8)
            nc.sync.dma_start(xa[:], xa_d[:])

            outb = small.tile([128, 24], F32)
            atb = small.tile([128, wt2, 2, RPC], F8)   # A^T (fp8)

            # ---- CE pieces (independent; DVE is idle at kernel start) ----
            for m in range(MT):
                nc.vector.reduce_max(outb[:, 18 + m:19 + m], pmt[:, m, :], axis=Ax.X)
                negmx = work.tile([128, 1], F32)
                nc.gpsimd.tensor_scalar_mul(negmx[:], outb[:, 18 + m:19 + m], -1.0)
                e7 = work.tile([128, C], F32)
                nc.scalar.activation(
                    e7[:], pmt[:, m, :], Act.Exp, bias=negmx[:, 0:1], scale=1.0,
                    accum_out=outb[:, 20 + m:21 + m],
                )

            # ---- NM = G - sq_j/2 - OFF*(1-same)  (fp8 DoubleRow) ----
            # both m-tiles' matmuls are emitted BEFORE any top-k consumer:
            # engine streams are executed in program order, so this keeps PE
            # grinding mm1(m1) while DVE runs m0's top-k chain
            nms = []
            for m in range(MT):
                mo = m * 128
                nm = pnm.tile([128, wcol], F32, tag="nm", bufs=2, name=f"nm{m}")
                nms.append(nm)
                for (s, n) in _chunks(wcol):
                    for k2 in range(KT2):
                        lhsT = (
                            rt[:, k2, :, mo:mo + 128]
                            if k2 < KT2 - 1
                            else lta[:, :, m * 128:(m + 1) * 128]
                        )
                        nc.tensor.matmul(
                            nm[:, s:s + n],
                            lhsT=lhsT,
                            rhs=rt[:, k2, :, s:s + n],
                            start=(k2 == 0),
                            stop=(k2 == KT2 - 1),
                            perf_mode=DR,
                        )

            mns = []
            v3s = []
            ige_insts = []
            for m in range(MT):
                ms = slice(m * 128, (m + 1) * 128)
                nm = nms[m]

                # SBUF copy of NM feeds the (later) is_ge + SNM reduce; the
                # top-k round 1 reads PSUM directly so the serial DVE chain
                # starts the moment the matmul group completes
                mn = work.tile([128, wcol], F32)
                nc.scalar.copy(mn[:], nm[:])
                mns.append(mn)

                # ---- top-(K+1) threshold: 3 rounds of max8 ----
                v1 = work.tile([128, 8], F32)
                mx = nc.vector.max(v1[:], nm[:])
                if m > 0:
                    # m1's chain strictly AFTER m0's threshold: the list
                    # scheduler otherwise interleaves the two serial DVE
                    # chains and m0's A-matrix lands ~5us late, stalling PE
                    add_dep_helper(mx.ins, ige_insts[m - 1].ins,
                                   reason="serialize m-tile top-k chains")
                mn2 = work.tile([128, wcol], F32)
                nc.vector.match_replace(mn2[:], v1[:], nm[:], NEG_FILL)
                v2 = work.tile([128, 8], F32)
                nc.vector.max(v2[:], mn2[:])
                mn3 = work.tile([128, wcol], F32)
                nc.vector.match_replace(mn3[:], v2[:], mn2[:], NEG_FILL)
                v3 = work.tile([128, 8], F32)
                nc.vector.max(v3[:], mn3[:])
                v3s.append(v3)

                # A = (NM >= t) as bf16 (ptr-scalar ops are DVE-only);
                # fp8 PE transpose needs stride-2 PSUM writes, so transpose
                # in bf16 and cast to fp8 in the PSUM->SBUF copy instead
                abh = work.tile([128, wcol], BF16)
                ige = nc.vector.tensor_scalar(
                    abh[:], mn[:], v3[:, 4:5], None, op0=Alu.is_ge)
                ige_insts.append(ige)
                for t2 in range(wt2):
                    tr = pnm.tile([128, 2, 128], BF16, tag="nm", bufs=2,
                                  name=f"tr{m}_{t2}")
                    for i in range(2):
                        t = 2 * t2 + i
                        nc.tensor.transpose(
                            tr[:, i], abh[:, t * 128:(t + 1) * 128], idt[:])
                    nc.scalar.copy(atb[:, t2, :, ms], tr[:])

                # ---- W' = A @ [x_w | split4(sq_w/32)]  (fp8 DoubleRow) ----
                # one single-bank PSUM tile per 512-chunk so each chunk's
                # matmul group is independent of the others' square-reduces
                for ci, (s, n) in enumerate(_chunks(NA)):
                    w = pw.tile([128, n], F32, tag=f"w{ci}", name=f"w{m}_{ci}")
                    for t2 in range(wt2):
                        nc.tensor.matmul(
                            w[:],
                            lhsT=atb[:, t2, :, ms],
                            rhs=xa[:, t2, :, s:s + n],
                            start=(t2 == 0),
                            stop=(t2 == wt2 - 1),
                            perf_mode=DR,
                        )
                    # pipelined ||s'||^2: square-reduce each chunk as soon
                    # as its group completes (exclude the sq cols). m0 on
                    # ACT Square; for m1 the DVE is free post-top-k, so ACT
                    # only downcasts PSUM->SBUF bf16 and DVE square-reduces
                    # -- this cuts the serial ACT tail after the last matmul
                    ne = min(s + n, D) - s
                    acc = outb[:, 10 + 4 * m + ci:11 + 4 * m + ci]
                    if m == 0:
                        sq2 = work.tile([128, 512], BF16, tag="sq2")
                        nc.scalar.activation(
                            sq2[:, :ne], w[:, :ne], Act.Square, accum_out=acc)
                    else:
                        wb = work.tile([128, 512], BF16, tag="sq2")
                        nc.scalar.copy(wb[:, :ne], w[:, :ne])
                        scr2 = work.tile([128, 512], BF16, tag="scr2")
                        nc.vector.scalar_tensor_tensor(
                            out=scr2[:, :ne], in0=wb[:, :ne], scalar=1.0,
                            in1=wb[:, :ne], op0=Alu.mult, op1=Alu.mult,
                            accum_out=acc)
                    if s + n > D:
                        lo = D - s
                        nc.scalar.copy(
                            outb[:, 2 + 4 * m:6 + 4 * m], w[:, lo:lo + NSQ]
                        )

            # deferred SNM reduces (off the critical top-k chain)
            for m in range(MT):
                scr = work.tile([128, wcol], F32)
                stt = nc.vector.scalar_tensor_tensor(
                    out=scr[:], in0=mns[m][:], scalar=v3s[m][:, 4:5],
                    in1=mns[m][:],
                    op0=Alu.is_ge, op1=Alu.mult,
                    accum_out=outb[:, m:m + 1],
                )
                # keep DVE on the m1 top-k chain until both A-matrices exist
                add_dep_helper(stt.ins, ige_insts[-1].ins,
                               reason="SNM reduces after last is_ge")

            nc.sync.dma_start(out_d[:], outb[:])

    nc.compile()
    return nc


def _plan_windows(ys):
    """Per-core window [ws, ws+wcol) covering the full class spans of the
    core's rows.  The window is later rolled so the core's own rows sit at
    columns [0, 256); only hi-lo <= wcol matters here, not alignment."""
    starts_c = np.searchsorted(ys, np.arange(C))
    ends_c = np.searchsorted(ys, np.arange(C), side="right")
    need = []
    for c in range(NCORES):
        blo, bhi = c * RPC, (c + 1) * RPC
        cls = np.unique(ys[blo:bhi])
        lo = int(min(starts_c[k] for k in cls))
        hi = int(max(ends_c[k] for k in cls))
        need.append((lo, hi))
    wneed = max(hi - lo for lo, hi in need)
    wcol = 256 * ((max(wneed, 512) + 255) // 256)  # even # of 128-tiles
    starts = []
    for c, (lo, hi) in enumerate(need):
        ws = min(lo, B - wcol)
        assert ws + wcol >= hi and 0 <= ws <= c * RPC, (c, ws, wcol, lo, hi)
        starts.append(ws)
    return wcol, starts


def kernel(preds, x, y):
    y = np.asarray(y).astype(np.int64)
    preds = np.ascontiguousarray(np.asarray(preds, dtype=np.float32))
    x = np.ascontiguousarray(np.asarray(x, dtype=np.float32))
    assert x.shape == (B, D) and preds.shape == (B, C) and y.shape == (B,)

    order = np.argsort(y, kind="stable")
    xs = x[order]
    ys = y[order]
    ps = preds[order]
    sq64 = np.einsum("ij,ij->i", xs.astype(np.float64), xs.astype(np.float64))
    sq = sq64.astype(np.float32)

    wcol, starts = _plan_windows(ys)
    cls_count = np.bincount(ys, minlength=C)
    assert (cls_count >= K + 1).all(), cls_count

    oh = np.zeros((C, B), np.float32)
    oh[ys, np.arange(B)] = 1.0

    # global augmented rhs for the NM matmul [KR, B] in e4m3:
    #   rows 0..D-1: x^T ; D..D+3: split4((-sq/2 - OFF)/32) with lhsT 32
    #   D+4..D+10: 8*one-hot(class) with lhsT 240*one-hot ; rest zero
    rhs_g = np.zeros((KR, B), NPF8)
    rhs_g[:D] = xs.T.astype(NPF8)
    qparts = _f8_split(-(sq64 / 2.0 + OFF), NSQ, SQS)
    for t in range(NSQ):
        rhs_g[D + t] = qparts[t]
    rhs_g[D + NSQ:D + NSQ + C] = (8.0 * oh).astype(NPF8)
    # partition-major swizzle [KR, B] -> [128, KT, B]
    rhs_gp = np.ascontiguousarray(rhs_g.reshape(KT, 128, B).transpose(1, 0, 2))

    xa_g = np.zeros((B, NA), NPF8)
    xa_g[:, :D] = xs.astype(NPF8)
    sparts = _f8_split(sq64, NSQ, SQS)
    for t in range(NSQ):
        xa_g[:, D + t] = sparts[t]

    if wcol not in _CACHE:
        _CACHE[wcol] = _build(wcol)
    nc = _CACHE[wcol]
    wt = wcol // 128

    # lta: lhsT for the LAST DoubleRow k-pair (k-tiles 14,15) of each core.
    # [p, 0, i] = feature row 1792+p of x_i ; [p, 1, i]: features 1920..1999
    # for p<80, then 32 (sq-split scale) for p in 80..83, then 240*one-hot.
    in_maps = []
    for cidx in range(NCORES):
        my = slice(cidx * RPC, (cidx + 1) * RPC)
        ws = starts[cidx]
        roll = cidx * RPC - ws   # my rows sit at window cols [roll, roll+256)
        lta = np.zeros((128, 2, RPC), NPF8)
        lta[:, 0, :] = xs[my, 1792:1920].T.astype(NPF8)
        lta[:80, 1, :] = xs[my, 1920:2000].T.astype(NPF8)
        lta[80:80 + NSQ, 1, :] = np.float32(SQS)
        lta[80 + NSQ:80 + NSQ + C, 1, :] = (240.0 * oh[:, my]).astype(NPF8)
        rtw = np.roll(rhs_gp[:, :, ws:ws + wcol], -roll, axis=2)
        xaw = np.roll(xa_g[ws:ws + wcol], -roll, axis=0)
        in_maps.append({
            "rt": np.ascontiguousarray(rtw.reshape(128, KT2, 2, wcol)),
            "lta": lta,
            "xa": np.ascontiguousarray(
                xaw.reshape(wt // 2, 2, 128, NA).transpose(2, 0, 1, 3)),
            "idt": np.eye(128, dtype=ml_dtypes.bfloat16),
            "pm": np.ascontiguousarray(
                ps[my].reshape(MT, 128, C).transpose(1, 0, 2)),
        })

    res = run_bass_kernel_spmd(nc, in_maps, core_ids=list(range(NCORES)))

    # host-side unshard: per-row stats -> two scalar loss terms
    lp_sum = 0.0
    ce_sum = 0.0
    for cidx in range(NCORES):
        my = slice(cidx * RPC, (cidx + 1) * RPC)
        o = res.results[cidx]["out"].astype(np.float64)
        sq_my = sq64[my]
        # snm_meas = sum_sel (G - sq_j/2);  sum_sel(-d2) = 2*snm - (K+1)*sq_i
        snm = 2.0 * np.stack([o[:, 0], o[:, 1]]).reshape(RPC) - (K + 1) * sq_my
        ssq = SQS * np.stack(
            [o[:, 2:6].sum(1), o[:, 6:10].sum(1)]).reshape(RPC)
        ssn = np.stack([o[:, 10:14].sum(1), o[:, 14:18].sum(1)]).reshape(RPC)
        mx = o[:, 18:20].T.reshape(RPC)
        se = o[:, 20:22].T.reshape(RPC)
        gp = 0.5 * (snm + (K + 1) * sq_my + ssq)
        lp = sq_my - (2.0 / K) * (gp - sq_my) + (ssn - 2.0 * gp + sq_my) / K**2
        lp_sum += lp.sum()
        lse = np.log(se) + mx
        pick = ps[my][np.arange(RPC), ys[my]].astype(np.float64)
        ce_sum += (lse - pick).sum()

    loss = LAMDA * (lp_sum / B) / 2.0 + ce_sum / B
    return np.float32(loss)
